# revision 7
# baseline (speedup 1.0000x reference)
"""Trainium2 Bass kernel for nn_AttentionBlock (B=8, S=2048, DIM_VAL=DIM_ATTN=512).

Sharding: pure data parallelism — batch element b runs on NeuronCore b (B=8 = n_cores).

Per-core dataflow (single batch element, S=2048, D=A=512):
  phase 0: DMA Wq/Wk/Wv, PE-transpose to WT[d, out] layout;
           DMA x [S, D] in 128-row tiles, PE-transpose to xT [D, S].
  phase 1: QT[a, s] = WqT.T @ xT   (a on partitions — ready to contract over a)
           KT[a, s] = WkT.T @ xT
           V [s, v] = xT-chunk.T @ WvT  (s on partitions — ready to contract over s)
  phase 2: per q-chunk of 512 query positions:
           ET[s_k, s_q] = exp((KT.T @ QT_chunk) * 1/sqrt(A))   # scores TRANSPOSED so
                                                               # ET feeds attn@V as lhsT
           r[s_q]      = ones[128,1].T @ ET  (rowsums on PE, accumulated over k-chunks)
           bounce r through DRAM to get it partition-aligned with U's rows
           U[s_q, v]   = ET.T @ V   ;   out = U * (1/r)  fused into PSUM->SBUF copy
Matmuls run as float32r (full-rate fp32 PE path); storage stays fp32.
"""

import os
import sys
from contextlib import ExitStack

import numpy as np

for _p in ("/root/.axon_site/_ro/trn_rl_repo", "/opt/trn_rl_repo"):
    if os.path.isdir(_p) and _p not in sys.path:
        sys.path.append(_p)

import concourse.bacc as bacc
import concourse.bass as bass
import concourse.mybir as mybir
import concourse.tile as tile
from concourse.bass_utils import run_bass_kernel_spmd
from concourse.masks import make_identity

B, S, D, A = 8, 2048, 512, 512
P = 128
N_CORES = 8
FP32 = mybir.dt.float32

# dtype used for matmul operand *views* (storage is fp32 either way).
# float32r = single-pass full-rate fp32 matmul path on TRN2's PE.
MM_DT = mybir.dt.float32r

SD = S // P        # 16 s-tiles of 128
DC = D // P        # 4 d-chunks of 128
AC = A // P        # 4 a-chunks of 128
NQ = 4             # q-chunks of 512
QW = S // NQ       # 512 — q-chunk width
KC = SD            # 16 k-chunks of 128


DEBUG_PHASE = "all"  # "p1" = stop after projections and dump QT/KT/V (debug only)


def build_body(ctx: ExitStack, tc: tile.TileContext, x_d, wq_d, wk_d, wv_d, out_d,
               dbg=None):
    nc = tc.nc

    const = ctx.enter_context(tc.tile_pool(name="const", bufs=1))
    identity = const.tile([P, P], FP32)
    make_identity(nc, identity[:])
    ones_f = const.tile([P, 1], FP32)
    nc.vector.memset(ones_f[:], 1.0)
    ones = const.tile([P, 1], MM_DT)
    nc.vector.tensor_copy(ones[:], ones_f[:])  # memset can't write fp32r

    # Persistent per-core tensors (live across both phases)
    persist = ctx.enter_context(tc.tile_pool(name="persist", bufs=1))
    QT = persist.tile([P, AC, S], MM_DT)     # [a%128, a//128, s]
    KT = persist.tile([P, AC, S], MM_DT)
    V = persist.tile([P, SD, D], MM_DT)      # [s%128, s//128, v]

    dram = ctx.enter_context(tc.tile_pool(name="dram", bufs=2, space="DRAM"))

    # ---------------- phase 0+1: transposes and projections ----------------
    with (
        tc.tile_pool(name="p1_sbuf", bufs=4) as p1,
        tc.tile_pool(name="p1_wt", bufs=1) as wtp,
        tc.tile_pool(name="p1_xt", bufs=1) as xtp,
        tc.tile_pool(name="ps_tp", bufs=4, space="PSUM") as ps_tp,
        tc.tile_pool(name="ps_mm", bufs=4, space="PSUM") as ps_mm,
    ):
        # --- weights: load natural [out, in], PE-transpose to [in, out] ---
        WTs = {}
        for name, w_d in (("wq", wq_d), ("wk", wk_d), ("wv", wv_d)):
            wt = wtp.tile([P, DC, 512], MM_DT, tag=f"wt_{name}")  # [d%128, d//128, out]
            WTs[name] = wt
            for oi in range(4):  # out-chunks of 128
                wnat = p1.tile([P, 512], FP32, tag="w_nat")
                nc.sync.dma_start(wnat[:], w_d[oi * P:(oi + 1) * P, :])
                for dj in range(DC):
                    pt = ps_tp.tile([P, P], FP32, tag="tp")
                    nc.tensor.transpose(pt[:], wnat[:, dj * P:(dj + 1) * P], identity[:])
                    nc.vector.tensor_copy(wt[:, dj, oi * P:(oi + 1) * P], pt[:])

        # --- x: load 128-row tiles, PE-transpose into xT [d%128, d//128, s] ---
        xT = xtp.tile([P, DC, S], MM_DT)
        for si in range(SD):
            xnat = p1.tile([P, D], FP32, tag="x_nat")
            nc.sync.dma_start(xnat[:], x_d[si * P:(si + 1) * P, :])
            for dj in range(DC):
                pt = ps_tp.tile([P, P], FP32, tag="tp")
                nc.tensor.transpose(pt[:], xnat[:, dj * P:(dj + 1) * P], identity[:])
                nc.vector.tensor_copy(xT[:, dj, si * P:(si + 1) * P], pt[:])

        # --- projections ---
        # QT[a, s] / KT[a, s]: lhsT = WT[d, a-chunk], rhs = xT[d, s-chunk(512)]
        for dst, wt in ((QT, WTs["wq"]), (KT, WTs["wk"])):
            for ai in range(AC):
                for sj in range(NQ):
                    pm = ps_mm.tile([P, QW], FP32, tag="proj")
                    for dk in range(DC):
                        nc.tensor.matmul(
                            pm[:],
                            (wt[:, dk, ai * P:(ai + 1) * P]),
                            (xT[:, dk, sj * QW:(sj + 1) * QW]),
                            start=(dk == 0),
                            stop=(dk == DC - 1),
                        )
                    nc.vector.tensor_copy(dst[:, ai, sj * QW:(sj + 1) * QW], pm[:])

        # V[s, v]: lhsT = xT[d, s-chunk(128)], rhs = WvT[d, v(512)]
        for si in range(SD):
            pm = ps_mm.tile([P, D], FP32, tag="proj")
            for dk in range(DC):
                nc.tensor.matmul(
                    pm[:],
                    (xT[:, dk, si * P:(si + 1) * P]),
                    (WTs["wv"][:, dk, :]),
                    start=(dk == 0),
                    stop=(dk == DC - 1),
                )
            nc.vector.tensor_copy(V[:, si, :], pm[:])

    if DEBUG_PHASE == "p1":
        assert dbg is not None
        qt_d, kt_d, v_d = dbg
        tmp = ctx.enter_context(tc.tile_pool(name="dbg", bufs=2))
        for ai in range(AC):
            t = tmp.tile([P, S], FP32, tag="dbgq")
            nc.vector.tensor_copy(t[:], QT[:, ai, :])
            nc.sync.dma_start(qt_d[ai * P:(ai + 1) * P, :], t[:])
            t2 = tmp.tile([P, S], FP32, tag="dbgk")
            nc.vector.tensor_copy(t2[:], KT[:, ai, :])
            nc.sync.dma_start(kt_d[ai * P:(ai + 1) * P, :], t2[:])
        for si in range(SD):
            t3 = tmp.tile([P, D], FP32, tag="dbgv")
            nc.vector.tensor_copy(t3[:], V[:, si, :])
            nc.sync.dma_start(v_d[si * P:(si + 1) * P, :], t3[:])
        return

    # ---------------- phase 2: attention, per q-chunk of 512 ----------------
    inv_sqrt_a = 1.0 / float(np.sqrt(A))
    with (
        tc.tile_pool(name="et", bufs=2) as etp,
        tc.tile_pool(name="p2_small", bufs=2) as p2s,
        tc.tile_pool(name="p2_out", bufs=3) as p2o,
        tc.tile_pool(name="ps_sc", bufs=4, space="PSUM") as ps_sc,
        tc.tile_pool(name="ps_r", bufs=2, space="PSUM") as ps_r,
        tc.tile_pool(name="ps_u", bufs=2, space="PSUM") as ps_u,
    ):
        for qc in range(NQ):
            qsl = slice(qc * QW, (qc + 1) * QW)

            # scores (transposed) + exp:  ET[s_k, s_q] per k-chunk
            et = etp.tile([P, KC, QW], MM_DT, tag="et")
            for ki in range(KC):
                pm = ps_sc.tile([P, QW], FP32, tag="sc")
                for ak in range(AC):
                    nc.tensor.matmul(
                        pm[:],
                        (KT[:, ak, ki * P:(ki + 1) * P]),
                        (QT[:, ak, qsl]),
                        start=(ak == 0),
                        stop=(ak == AC - 1),
                    )
                nc.scalar.activation(
                    et[:, ki, :], pm[:],
                    mybir.ActivationFunctionType.Exp,
                    scale=inv_sqrt_a,
                )

            # rowsums r[s_q] = sum_k ET[k, s_q]  (ones as stationary operand)
            pr = ps_r.tile([1, QW], FP32, tag="r")
            for ki in range(KC):
                nc.tensor.matmul(
                    pr[:],
                    (ones[:]),
                    (et[:, ki, :]),
                    start=(ki == 0),
                    stop=(ki == KC - 1),
                )
            r_sb = p2s.tile([1, QW], FP32, tag="r_sb")
            nc.vector.tensor_copy(r_sb[:], pr[:])
            # bounce through DRAM to realign: r[128 rows of this q-chunk, 4 subchunks]
            r_dram = dram.tile([1, QW], FP32, tag="r_dram")
            nc.sync.dma_start(r_dram[:], r_sb[:])
            r_part = p2s.tile([P, NQ], FP32, tag="r_part")
            nc.sync.dma_start(
                r_part[:], r_dram[:].rearrange("one (c p) -> (one p) c", p=P)
            )
            rinv = p2s.tile([P, NQ], FP32, tag="rinv")
            nc.vector.reciprocal(rinv[:], r_part[:])

            # U = ET.T @ V per 128-row output chunk; divide fused into copy-out
            for c in range(NQ):
                pu = ps_u.tile([P, D], FP32, tag="u")
                for ki in range(KC):
                    nc.tensor.matmul(
                        pu[:],
                        (et[:, ki, c * P:(c + 1) * P]),
                        (V[:, ki, :]),
                        start=(ki == 0),
                        stop=(ki == KC - 1),
                    )
                o_sb = p2o.tile([P, D], FP32, tag="o_sb")
                nc.vector.tensor_scalar_mul(o_sb[:], pu[:], rinv[:, c:c + 1])
                row0 = qc * QW + c * P
                nc.sync.dma_start(out_d[row0:row0 + P, :], o_sb[:])


def build_program() -> bass.Bass:
    nc = bacc.Bacc("TRN2", target_bir_lowering=False, debug=False,
                   num_devices=N_CORES)
    x_d = nc.dram_tensor("x", [S, D], FP32, kind="ExternalInput").ap()
    wq_d = nc.dram_tensor("Wq", [A, D], FP32, kind="ExternalInput").ap()
    wk_d = nc.dram_tensor("Wk", [A, D], FP32, kind="ExternalInput").ap()
    wv_d = nc.dram_tensor("Wv", [D, D], FP32, kind="ExternalInput").ap()
    out_d = nc.dram_tensor("out", [S, D], FP32, kind="ExternalOutput").ap()
    dbg = None
    if DEBUG_PHASE == "p1":
        dbg = (nc.dram_tensor("qt", [A, S], FP32, kind="ExternalOutput").ap(),
               nc.dram_tensor("kt", [A, S], FP32, kind="ExternalOutput").ap(),
               nc.dram_tensor("v", [S, D], FP32, kind="ExternalOutput").ap())
    with tile.TileContext(nc) as tc:
        with ExitStack() as ctx:
            build_body(ctx, tc, x_d, wq_d, wk_d, wv_d, out_d, dbg=dbg)
    nc.compile()
    return nc


_prog_cache = {}


def _get_program() -> bass.Bass:
    if "nc" not in _prog_cache:
        _prog_cache["nc"] = build_program()
    return _prog_cache["nc"]


def make_in_maps(x, Wq, Wk, Wv):
    x = np.ascontiguousarray(np.asarray(x), dtype=np.float32)
    Wq = np.ascontiguousarray(np.asarray(Wq), dtype=np.float32)
    Wk = np.ascontiguousarray(np.asarray(Wk), dtype=np.float32)
    Wv = np.ascontiguousarray(np.asarray(Wv), dtype=np.float32)
    return [
        {"x": x[i], "Wq": Wq, "Wk": Wk, "Wv": Wv} for i in range(N_CORES)
    ]


def run_spmd(x, Wq, Wk, Wv, **kw):
    nc = _get_program()
    return run_bass_kernel_spmd(nc, make_in_maps(x, Wq, Wk, Wv),
                                list(range(N_CORES)), **kw)


def kernel(x, Wq, Wk, Wv):
    res = run_spmd(x, Wq, Wk, Wv)
    return np.stack([res.results[i]["out"] for i in range(N_CORES)], axis=0)
